# revision 11
# baseline (speedup 1.0000x reference)
"""Trainium2 Bass kernel for MiniGPT4 CAM sparse attention.

Sharding: tensor-parallel over 16 heads -> 2 heads per core (8 cores).
Each core: q/k/v projections for its 2 heads (k/v only at the CAM-allowed
kv positions: [0,410) + [3072,4096)), RoPE, attention with the softmax
denominator folded into an augmented-V matmul (ones column), and a
row-sharded output projection producing a partial [2048, 4096] final^T.
Partials are summed on the host (gather/unshard step).

Matmuls run in bf16 (full-rate on the PE); accumulation is fp32 in PSUM,
softmax/rope arithmetic is fp32.
"""

import math
import numpy as np

HID = 2048
HEADS = 16
HEAD_DIM = 128
SEQ = 4096
N_CORES = 8
HPC = HEADS // N_CORES          # heads per core = 2
DLOC = HPC * HEAD_DIM           # 256 local head dims
START = math.ceil(0.1 * SEQ)    # 410
RECENT = math.ceil(0.25 * SEQ)  # 1024
KV_HI = SEQ - RECENT            # 3072
NKV = START + RECENT            # 1434
SHIFT = 15.0                    # global exp shift (cancels in softmax)

P = 128
N_STILES = SEQ // P             # 32 seq subtiles
N_HTILES = HID // P             # 16 hidden tiles
QB = 512                        # q block (free dim of scores matmuls)
N_QB = SEQ // QB                # 8

# kv subtiles: (seq_subtile_index, rows_used) covering [0,410) + [3072,4096)
KV_STILES = [(0, 128), (1, 128), (2, 128), (3, 26)] + [(24 + i, 128) for i in range(8)]
N_KVT = len(KV_STILES)          # 12
# column offsets of each kv tile inside the compacted [*,1434] layout
KV_COL = np.cumsum([0] + [r for _, r in KV_STILES]).tolist()
NKV_PAD = 1440                  # 1434 padded a touch for tile allocation

_CACHE = {}


def _build_program(mm="bf16"):
    import concourse.bass as bass
    import concourse.bacc as bacc
    import concourse.mybir as mybir
    from concourse.tile import TileContext
    from contextlib import ExitStack

    fp32 = mybir.dt.float32
    bf16 = mybir.dt.bfloat16
    AF = mybir.ActivationFunctionType
    ALU = mybir.AluOpType

    nc = bacc.Bacc()

    hs_d = nc.declare_dram_parameter("hs", [SEQ, HID], fp32, isOutput=False)
    wq_d = nc.declare_dram_parameter("wq", [HID, DLOC], fp32, isOutput=False)
    wk_d = nc.declare_dram_parameter("wk", [HID, DLOC], fp32, isOutput=False)
    wv_d = nc.declare_dram_parameter("wv", [HID, DLOC], fp32, isOutput=False)
    wo_d = nc.declare_dram_parameter("wo", [DLOC, HID], fp32, isOutput=False)
    bq_d = nc.declare_dram_parameter("bq", [1, DLOC], fp32, isOutput=False)
    bk_d = nc.declare_dram_parameter("bk", [1, DLOC], fp32, isOutput=False)
    bv_d = nc.declare_dram_parameter("bv", [1, DLOC], fp32, isOutput=False)
    bo_d = nc.declare_dram_parameter("bo", [1, HID], fp32, isOutput=False)
    cq_d = nc.declare_dram_parameter("cq", [N_STILES, P, 64], fp32, isOutput=False)
    sq_d = nc.declare_dram_parameter("sq", [N_STILES, P, 64], fp32, isOutput=False)
    ck_d = nc.declare_dram_parameter("ck", [N_STILES, P, 64], fp32, isOutput=False)
    sk_d = nc.declare_dram_parameter("sk", [N_STILES, P, 64], fp32, isOutput=False)
    id_d = nc.declare_dram_parameter("ident", [P, P], fp32, isOutput=False)
    out_d = nc.declare_dram_parameter("pout", [HID, SEQ], fp32, isOutput=True)

    with TileContext(nc) as tc, ExitStack() as top:
        const = top.enter_context(tc.tile_pool(name="const", bufs=1))
        persist = top.enter_context(tc.tile_pool(name="persist", bufs=1))
        tpsum = top.enter_context(tc.tile_pool(name="tpsum", bufs=2, space="PSUM"))

        # ---- constants (bf16 matmul operands via SWDGE cast DMA) ----
        wq_sb = const.tile([P, N_HTILES, DLOC], bf16)
        wk_sb = const.tile([P, N_HTILES, DLOC], bf16)
        wv_sb = const.tile([P, N_HTILES, DLOC], bf16)
        nc.gpsimd.dma_start(wq_sb[:], wq_d[:].rearrange("(t p) d -> p t d", p=P))
        nc.gpsimd.dma_start(wk_sb[:], wk_d[:].rearrange("(t p) d -> p t d", p=P))
        nc.gpsimd.dma_start(wv_sb[:], wv_d[:].rearrange("(t p) d -> p t d", p=P))
        bq_sb = const.tile([1, DLOC], bf16)
        bk_sb = const.tile([1, DLOC], bf16)
        bv_sb = const.tile([1, DLOC], bf16)
        bo_sb = const.tile([1, HID], bf16)
        nc.gpsimd.dma_start(bq_sb[:], bq_d[:])
        nc.gpsimd.dma_start(bk_sb[:], bk_d[:])
        nc.gpsimd.dma_start(bv_sb[:], bv_d[:])
        nc.gpsimd.dma_start(bo_sb[:], bo_d[:])
        ident_sb = const.tile([P, P], bf16)
        nc.gpsimd.dma_start(ident_sb[:], id_d[:])
        cq_sb = const.tile([P, N_STILES, 64], fp32)
        sq_sb = const.tile([P, N_STILES, 64], fp32)
        nc.sync.dma_start(cq_sb[:], cq_d[:].rearrange("t p d -> p t d"))
        nc.sync.dma_start(sq_sb[:], sq_d[:].rearrange("t p d -> p t d"))
        ones_sb = const.tile([1, QB], bf16)
        nc.gpsimd.memset(ones_sb[:], 1.0)
        shift_sb = const.tile([P, 1], fp32)
        nc.gpsimd.memset(shift_sb[:], -SHIFT)

        # ---- persistent activations ----
        qT_sb = persist.tile([P, HPC, SEQ], bf16)       # q^T per head [128d, 4096]
        kT_sb = persist.tile([P, HPC, NKV_PAD], bf16)   # k^T per head [128d, 1434]
        vA_sb = persist.tile([P, HPC, N_KVT, 132], bf16)  # v_aug per head/kvtile
        oT_sb = persist.tile([P, HPC, SEQ], bf16)       # O^T per head (normalized)

        # ones column of v_aug (col 128); cols 129..131 payload-free padding
        for h in range(HPC):
            for t in range(N_KVT):
                nc.gpsimd.memset(vA_sb[:, h, t, 128:129], 1.0)

        # ================= stage A: transpose + projections + rope ==========
        with ExitStack() as sa:
            hsp = sa.enter_context(tc.tile_pool(name="hsp", bufs=3))
            hstp = sa.enter_context(tc.tile_pool(name="hstp", bufs=3))
            ropep = sa.enter_context(tc.tile_pool(name="ropep", bufs=3))
            cksl = sa.enter_context(tc.tile_pool(name="cksl", bufs=2))
            ppsum = sa.enter_context(tc.tile_pool(name="ppsum", bufs=2, space="PSUM"))

            kv_of_stile = {st: (i, rows) for i, (st, rows) in enumerate(KV_STILES)}

            for st in range(N_STILES):
                hs_t = hsp.tile([P, HID], bf16, tag="hs")
                nc.gpsimd.dma_start(hs_t[:], hs_d[st * P:(st + 1) * P, :])

                # transpose hs subtile -> hsT [128h x (16,128s)]
                hsT = hstp.tile([P, N_HTILES, P], bf16, tag="hsT")
                for g in range(N_HTILES // 4):
                    ps = tpsum.tile([P, 4 * P], bf16, tag="tp")
                    for j in range(4):
                        ht = g * 4 + j
                        nc.tensor.transpose(
                            ps[:, j * P:(j + 1) * P],
                            hs_t[:, ht * P:(ht + 1) * P], ident_sb[:])
                    nc.vector.tensor_copy(
                        hsT[:, g * 4:(g + 1) * 4, :].rearrange("p a b -> p (a b)"),
                        ps[:])

                # ---- q projection: out [128s, 256d] ----
                qp = ppsum.tile([P, DLOC], fp32, tag="qp")
                nc.tensor.matmul(qp[:], ones_sb[:, 0:P], bq_sb[:],
                                 start=True, stop=False)
                for ht in range(N_HTILES):
                    nc.tensor.matmul(qp[:], hsT[:, ht, :], wq_sb[:, ht, :],
                                     start=False, stop=(ht == N_HTILES - 1))

                # rope q (both heads batched via 2-chunk APs), scaled tables
                qr = ropep.tile([P, DLOC], bf16, tag="qr")
                tmp = ropep.tile([P, P], fp32, tag="tmp")

                def rope(dst, src, cos_ap, sin_ap, rows=P):
                    # per head: x1 = src[:, h*128+0:64], x2 = src[:, h*128+64:128]
                    sr = src[0:rows, :].rearrange("p (h x d) -> p h x d", h=HPC, x=2)
                    dr = dst[0:rows, :].rearrange("p (h x d) -> p h x d", h=HPC, x=2)
                    x1, x2 = sr[:, :, 0, :], sr[:, :, 1, :]
                    o1, o2 = dr[:, :, 0, :], dr[:, :, 1, :]
                    cb = cos_ap[0:rows, :].rearrange("p (o d) -> p o d", o=1).broadcast_to([rows, HPC, 64])
                    sb_ = sin_ap[0:rows, :].rearrange("p (o d) -> p o d", o=1).broadcast_to([rows, HPC, 64])
                    tr = tmp[0:rows, :].rearrange("p (h d) -> p h d", h=HPC)
                    # o1 = x1*c - x2*s ; o2 = x1*s + x2*c
                    nc.vector.tensor_tensor(o1, x1, cb, ALU.mult)
                    nc.vector.tensor_tensor(tr, x2, sb_, ALU.mult)
                    nc.vector.tensor_tensor(o1, o1, tr, ALU.subtract)
                    nc.vector.tensor_tensor(o2, x1, sb_, ALU.mult)
                    nc.vector.tensor_tensor(tr, x2, cb, ALU.mult)
                    nc.vector.tensor_tensor(o2, o2, tr, ALU.add)

                rope(qr, qp, cq_sb[:, st, :], sq_sb[:, st, :])

                # transpose roped q into qT
                for h in range(HPC):
                    ps = tpsum.tile([P, 4 * P], bf16, tag="tp")
                    nc.tensor.transpose(ps[:, 0:P], qr[:, h * P:(h + 1) * P], ident_sb[:])
                    nc.vector.tensor_copy(qT_sb[:, h, st * P:(st + 1) * P], ps[:, 0:P])

                # ---- k/v projections on kv subtiles ----
                if st in kv_of_stile:
                    ti, rows = kv_of_stile[st]
                    col = KV_COL[ti]
                    ck_t = cksl.tile([P, 64], fp32, tag="ck")
                    sk_t = cksl.tile([P, 64], fp32, tag="sk")
                    nc.sync.dma_start(ck_t[:rows, :], ck_d[st, 0:rows, :])
                    nc.sync.dma_start(sk_t[:rows, :], sk_d[st, 0:rows, :])

                    kp = ppsum.tile([P, DLOC], fp32, tag="qp")
                    nc.tensor.matmul(kp[0:rows, :], ones_sb[:, 0:rows], bk_sb[:],
                                     start=True, stop=False)
                    for ht in range(N_HTILES):
                        nc.tensor.matmul(kp[0:rows, :], hsT[:, ht, 0:rows],
                                         wk_sb[:, ht, :],
                                         start=False, stop=(ht == N_HTILES - 1))
                    kr = ropep.tile([P, DLOC], bf16, tag="qr")
                    rope(kr, kp, ck_t, sk_t, rows=rows)

                    for h in range(HPC):
                        ps = tpsum.tile([P, 4 * P], bf16, tag="tp")
                        nc.tensor.transpose(ps[0:P, 0:rows],
                                            kr[0:rows, h * P:(h + 1) * P],
                                            ident_sb[0:rows, 0:rows])
                        nc.vector.tensor_copy(kT_sb[:, h, col:col + rows], ps[:, 0:rows])

                    vp = ppsum.tile([P, DLOC], fp32, tag="qp")
                    nc.tensor.matmul(vp[0:rows, :], ones_sb[:, 0:rows], bv_sb[:],
                                     start=True, stop=False)
                    for ht in range(N_HTILES):
                        nc.tensor.matmul(vp[0:rows, :], hsT[:, ht, 0:rows],
                                         wv_sb[:, ht, :],
                                         start=False, stop=(ht == N_HTILES - 1))
                    for h in range(HPC):
                        nc.vector.tensor_copy(vA_sb[0:rows, h, ti, 0:P],
                                              vp[0:rows, h * P:(h + 1) * P])

        # ================= stage B: attention =================
        with ExitStack() as sb:
            expp = sb.enter_context(tc.tile_pool(name="expp", bufs=2))
            normp = sb.enter_context(tc.tile_pool(name="normp", bufs=3))
            spsum = sb.enter_context(tc.tile_pool(name="spsum", bufs=2, space="PSUM"))
            apsum = sb.enter_context(tc.tile_pool(name="apsum", bufs=2, space="PSUM"))

            for h in range(HPC):
                for qb in range(N_QB):
                    ex = expp.tile([P, N_KVT, QB], bf16, tag="exp")
                    for ti, (st, rows) in enumerate(KV_STILES):
                        col = KV_COL[ti]
                        sp = spsum.tile([P, QB], fp32, tag="sc")
                        nc.tensor.matmul(
                            sp[0:rows, :],
                            kT_sb[:, h, col:col + rows],
                            qT_sb[:, h, qb * QB:(qb + 1) * QB],
                            start=True, stop=True)
                        nc.scalar.activation(ex[0:rows, ti, :], sp[0:rows, :],
                                             AF.Exp, bias=shift_sb[0:rows, :],
                                             scale=1.0)
                    for qt in range(QB // P):
                        op = apsum.tile([P, 132], fp32, tag="ov")
                        for ti, (st, rows) in enumerate(KV_STILES):
                            nc.tensor.matmul(
                                op[:, 0:129],
                                ex[0:rows, ti, qt * P:(qt + 1) * P],
                                vA_sb[0:rows, h, ti, 0:129],
                                start=(ti == 0), stop=(ti == N_KVT - 1))
                        recip = normp.tile([P, 1], fp32, tag="recip")
                        nc.vector.reciprocal(recip[:], op[:, 128:129])
                        onorm = normp.tile([P, P], bf16, tag="onorm")
                        nc.vector.tensor_scalar_mul(onorm[:], op[:, 0:P], recip[:])
                        ps = tpsum.tile([P, 4 * P], bf16, tag="tp")
                        nc.tensor.transpose(ps[:, 0:P], onorm[:], ident_sb[:])
                        q0 = qb * QB + qt * P
                        nc.vector.tensor_copy(oT_sb[:, h, q0:q0 + P], ps[:, 0:P])

        # ================= stage C: output projection (row-sharded) =========
        with ExitStack() as sc:
            wop = sc.enter_context(tc.tile_pool(name="wop", bufs=1))
            outp = sc.enter_context(tc.tile_pool(name="outp", bufs=2))
            opsum = sc.enter_context(tc.tile_pool(name="opsum", bufs=2, space="PSUM"))

            wo_sb = wop.tile([P, HPC, HID], bf16)
            nc.gpsimd.dma_start(wo_sb[:], wo_d[:].rearrange("(t p) e -> p t e", p=P))

            for et in range(N_HTILES):
                stage = outp.tile([P, SEQ], fp32, tag="stage")
                for qb in range(N_QB):
                    fp = opsum.tile([P, QB], fp32, tag="fp")
                    nc.tensor.matmul(fp[:], bo_sb[:, et * P:(et + 1) * P],
                                     ones_sb[:], start=True, stop=False)
                    for h in range(HPC):
                        nc.tensor.matmul(
                            fp[:],
                            wo_sb[:, h, et * P:(et + 1) * P],
                            oT_sb[:, h, qb * QB:(qb + 1) * QB],
                            start=False, stop=(h == HPC - 1))
                    nc.scalar.copy(stage[:, qb * QB:(qb + 1) * QB], fp[:])
                nc.sync.dma_start(out_d[et * P:(et + 1) * P, :], stage[:])

    nc.finalize()
    return nc


def _host_inputs(inputs):
    hs = np.ascontiguousarray(np.asarray(inputs["hidden_states"], np.float32).reshape(SEQ, HID))
    Wq = np.asarray(inputs["Wq"], np.float32)
    Wk = np.asarray(inputs["Wk"], np.float32)
    Wv = np.asarray(inputs["Wv"], np.float32)
    Wo = np.asarray(inputs["Wo"], np.float32)
    bq = np.asarray(inputs["bq"], np.float32)
    bk = np.asarray(inputs["bk"], np.float32)
    bv = np.asarray(inputs["bv"], np.float32)
    bo = np.asarray(inputs["bo"], np.float32)

    theta = 1.0 / (10000.0 ** (np.arange(0, HEAD_DIM, 2, dtype=np.float32) / HEAD_DIM))
    sinusoid = np.arange(SEQ, dtype=np.float32)[:, None] * theta[None, :]
    sin = np.sin(sinusoid).astype(np.float32)
    cos = np.cos(sinusoid).astype(np.float32)
    scale = np.float32(1.0 / math.sqrt(HEAD_DIM))
    cq = (cos * scale).reshape(N_STILES, P, 64)
    sq = (sin * scale).reshape(N_STILES, P, 64)
    ck = cos.reshape(N_STILES, P, 64)
    sk = sin.reshape(N_STILES, P, 64)
    ident = np.eye(P, dtype=np.float32)

    perm = np.concatenate([np.arange(0, HEAD_DIM, 2), np.arange(1, HEAD_DIM, 2)])
    in_maps = []
    for c in range(N_CORES):
        cols_pk = np.concatenate([c * 256 + h * 128 + perm for h in range(HPC)])
        sl = slice(c * 256, (c + 1) * 256)
        in_maps.append({
            "hs": hs,
            "wq": np.ascontiguousarray(Wq.T[:, cols_pk]),
            "wk": np.ascontiguousarray(Wk.T[:, cols_pk]),
            "wv": np.ascontiguousarray(Wv.T[:, sl]),
            "wo": np.ascontiguousarray(Wo.T[sl, :]),
            "bq": np.ascontiguousarray(bq[cols_pk]).reshape(1, DLOC),
            "bk": np.ascontiguousarray(bk[cols_pk]).reshape(1, DLOC),
            "bv": np.ascontiguousarray(bv[sl]).reshape(1, DLOC),
            "bo": np.ascontiguousarray(bo / N_CORES).reshape(1, HID),
            "cq": cq, "sq": sq, "ck": ck, "sk": sk,
            "ident": ident,
        })
    return in_maps


def run(inputs, trace=False, mm="bf16"):
    from concourse.bass_utils import run_bass_kernel_spmd
    key = mm
    if key not in _CACHE:
        _CACHE[key] = _build_program(mm)
    nc = _CACHE[key]
    in_maps = _host_inputs(inputs)
    res = run_bass_kernel_spmd(nc, in_maps, core_ids=list(range(N_CORES)),
                               trace=trace)
    acc = np.zeros((HID, SEQ), np.float64)
    for r in res.results:
        acc += r["pout"].astype(np.float64)
    out = acc.T.astype(np.float32).reshape(1, SEQ, HID)
    return out, res


def kernel(**inputs) -> np.ndarray:
    out, _ = run(inputs, trace=False)
    return out
